# revision 61
# baseline (speedup 1.0000x reference)
"""DiT attention kernel for Trainium2 (Bass/Tile), data-parallel over batch.

Problem: B=8, S=1024, D=1024, H=16 heads, head_dim=64, fp32.
  q = x@wq.T; k = x@wk.T; v = x@wv.T          (per batch)
  attn = softmax(q k^T / sqrt(hd)); out = (attn v) @ wo.T

Sharding: batch is split 1:1 onto the 8 NeuronCores (pure data parallel,
no collectives). Weights are broadcast. Host pre-transposes x (per batch)
and the four weights so every matmul has its contraction dim on SBUF
partitions; all matmuls run as float32r (full-rate fp32, ~1e-4 rel err).

Per-core dataflow (everything [part, free] in SBUF):
  xT   [d, s]    : DMA (host-transposed input)
  Q^T  [o, s]    : lhsT=wqT column chunk, rhs=xT          (per o-chunk)
  K^T  [o, s]    : same with wkT
  V    [s, o]    : lhsT=xT chunk, rhs=wvT row tiles, stored per-head with
                   an appended ones column (V_aug [s, h, 65]) so the attnV
                   matmul also produces the softmax denominator.
  per head h:    S^T[k,q] = K_h^T chunkT @ Q_h^T (K=64), exp on ACT
                 (scale=1/8 folded in, no max-subtraction: scores ~N(0,1)),
                 raw^T[hd+1, q] = V_aug^T @ expS^T accumulated over k.
  softmax denom rows of a head pair are collected into a [32, q] tile via
  SBUF->SBUF DMA (partition shift), reciprocal'd, broadcast back across
  the pair's two 64-partition bands with a K=32 selector matmul, and
  multiplied into raw^T — all lagged one pair so PE never waits.
  Y[s, o] = lhsT=rawT chunk, rhs=woT row tiles -> DMA out.

Scheduling: Q/K projections for chunk oc+1 are emitted as 8-piece fillers
interleaved into chunk oc's head kc-loops (heads alone are ACT-rate-bound
by exp); attnV lags exp by one kc; pair normalization lags one pair and
uses reciprocal_approx_fast (HW DIVIDE runs 8 cycles/elem; the approx op
runs at line rate at ~2e-6 rel err). Cost-model time ~271.5us/core at
~85% PE occupancy; measured relative error ~4e-4 (float32r is a
reduced-mantissa fp32 matmul mode).
"""
import numpy as np
from contextlib import ExitStack

import concourse.bass as bass
import concourse.mybir as mybir
import concourse.tile as tile
from concourse import bacc
import concourse.bass_utils as bass_utils
from concourse.bass import ds

B, S, D, H = 8, 1024, 1024, 16
HD = D // H          # 64
P = 128
NCORES = 8
DC = D // P          # 8 chunks of the feature dim
SC = S // P          # 8 chunks of the sequence dim
NH = 512             # matmul moving-dim chunk (fp32 limit, one PSUM bank)

f32 = mybir.dt.float32
f32r = mybir.dt.float32r
AF = mybir.ActivationFunctionType
ALU = mybir.AluOpType


def emit(tc, xT_d, wqT_d, wkT_d, wvT_d, woT_d, y_d):
    nc = tc.nc
    with ExitStack() as ctx:
        xp = ctx.enter_context(tc.tile_pool(name="xp", bufs=1))
        qkp = ctx.enter_context(tc.tile_pool(name="qkp", bufs=1))
        vp = ctx.enter_context(tc.tile_pool(name="vp", bufs=1))
        ep = ctx.enter_context(tc.tile_pool(name="ep", bufs=4))
        rp = ctx.enter_context(tc.tile_pool(name="rp", bufs=1))
        stp = ctx.enter_context(tc.tile_pool(name="stp", bufs=1))
        sxq = ctx.enter_context(tc.tile_pool(name="sxq", bufs=2))
        sxp = ctx.enter_context(tc.tile_pool(name="sxp", bufs=1))
        wp = ctx.enter_context(tc.tile_pool(name="wp", bufs=3))
        wrp = ctx.enter_context(tc.tile_pool(name="wrp", bufs=3))
        yp = ctx.enter_context(tc.tile_pool(name="yp", bufs=2))
        pp = ctx.enter_context(tc.tile_pool(name="pp", bufs=4, space="PSUM"))

        # ---- V projection: V_aug [s_part, sc, head, 65] ----
        # xT tiles are loaded just-in-time inside the first V pass so the
        # first matmul only waits for xT[0] + wv[0] (not the full 4MB of x)
        V = vp.tile([P, SC, H, HD + 1], f32r, tag="v")
        ones_t = yp.tile([P, H], f32, tag="y")
        nc.vector.memset(ones_t[:], 1.0)
        for sc in range(SC):
            nc.vector.tensor_copy(V[:, sc, :, HD], ones_t[:])
        def load_wqk(oc, key, wd):
            wt = wp.tile([P, DC, P], f32r, tag="wqk", name=f"w{key}{oc}")
            # wq/wk are host-blocked to [oc, p, dc, o]: this load is one DMA
            # of 128 contiguous 4KB descriptors
            nc.sync.dma_start(wt[:], wd[oc])
            return wt

        xts = []

        def emit_v_pass(oh):
            psVs = [pp.tile([P, 2 * NH], f32, tag="ps", name=f"psV{oh}_{j}") for j in range(4)]
            for dc in range(DC):
                wvt = wrp.tile([P, NH], f32r, tag="wr")
                nc.sync.dma_start(wvt[:], wvT_d[ds(dc * P, P), ds(oh * NH, NH)])
                if oh == 0:
                    t = xp.tile([P, S], f32r, tag=f"x{dc}")
                    # two halves: the first V matmul only waits for 256KB
                    nc.sync.dma_start(t[:, 0:NH], xT_d[ds(dc * P, P), 0:NH])
                    nc.sync.dma_start(t[:, NH:S], xT_d[ds(dc * P, P), NH:S])
                    xts.append(t)
                for sc in range(SC):
                    nc.tensor.matmul(
                        psVs[sc // 2][:, ds((sc % 2) * NH, NH)],
                        xts[dc][:, ds(sc * P, P)], wvt[:],
                        start=(dc == 0), stop=(dc == DC - 1))
            for sc in range(SC):
                src = psVs[sc // 2][:, ds((sc % 2) * NH, NH)]
                dst = V[:, sc, ds(oh * 8, 8), 0:HD]
                if sc % 2 == 0:
                    nc.vector.tensor_copy(dst, src.rearrange("p (h e) -> p h e", e=HD))
                else:
                    nc.scalar.copy(dst, src.rearrange("p (h e) -> p h e", e=HD))

        emit_v_pass(0)
        emit_v_pass(1)

        # ---- softmax-denominator spread selector ----
        # sel2[k, p2, m] = (k == p2): K=32-padded lhsT that broadcasts the
        # two sumexp rows of a head pair across the 2x64 partition bands.
        # Built in a transient f32 tile (borrowed wp slot), then DVE-copied
        # to f32r so the matmul operand has a rounding producer.
        sel2_f = wp.tile([2 * H, P], f32, tag="wqk")
        nc.vector.memset(sel2_f[:], 1.0)
        nc.gpsimd.affine_select(
            out=sel2_f[:].rearrange("k (p2 m) -> k p2 m", m=HD),
            in_=sel2_f[:].rearrange("k (p2 m) -> k p2 m", m=HD),
            compare_op=ALU.is_equal,
            fill=0.0,
            base=0,
            pattern=[[-1, 2], [0, HD]],
            channel_multiplier=1,
        )
        sel2 = sxp.tile([2 * H, P], f32r, tag="on")
        nc.vector.tensor_copy(sel2[:], sel2_f[:])

        # ---- software-pipelined Q/K projection + attention ----
        # Q/K for chunk oc+1 are emitted between the two heads of chunk oc,
        # so the scores of a head never wait on a drain that just ran.
        QT, KT, raws = {}, {}, {}

        def qk_gen(oc, key, wd, store, wt=None):
            """Generator: emits the oc-chunk Q/K projection in 8 pieces so it
            can be interleaved into an attention head's kc loop as PE filler
            (the head alone is ACT-rate-limited by exp)."""
            if wt is None:
                wt = load_wqk(oc, key, wd)
            ps = pp.tile([P, 2 * NH], f32, tag="ps", name=f"ps{key}{oc}")
            for dc in range(DC):
                for sh in range(2):
                    nc.tensor.matmul(
                        ps[:, ds(sh * NH, NH)], wt[:, dc, :],
                        xts[dc][:, ds(sh * NH, NH)],
                        start=(dc == 0), stop=(dc == DC - 1))
                yield
            dst = qkp.tile([P, S], f32r, tag=f"{key}{oc}", name=f"t{key}{oc}")
            nc.vector.tensor_copy(dst[:], ps[:])
            store[oc] = dst

        def emit_qk(oc, key, wd, store, wt=None):
            for _ in qk_gen(oc, key, wd, store, wt=wt):
                pass

        def emit_head(oc, hh, rawt, sxpair, filler=None):
            h = 2 * oc + hh
            psO = pp.tile([P, 2 * NH], f32, tag="ps", name=f"psO{h}")
            ets = {}

            def attn_v(kc):
                for qh in range(2):
                    nc.tensor.matmul(
                        psO[0:HD + 1, ds(qh * NH, NH)],
                        V[:, kc, h, :], ets[kc][:, ds(qh * NH, NH)],
                        start=(kc == 0), stop=(kc == SC - 1))

            # attnV is emitted one kc behind exp so PE never stalls on ACT
            for kc in range(SC):
                psS = pp.tile([P, 2 * NH], f32, tag="ps", name=f"psS{h}_{kc}")
                lhsT = KT[oc][ds(hh * HD, HD), ds(kc * P, P)]
                for qh in range(2):
                    nc.tensor.matmul(
                        psS[:, ds(qh * NH, NH)], lhsT,
                        QT[oc][ds(hh * HD, HD), ds(qh * NH, NH)],
                        start=True, stop=True)
                et = ep.tile([P, S], f32r, tag="e", name=f"et{h}_{kc}")
                nc.scalar.activation(et[:], psS[:], AF.Exp, scale=0.125)
                ets[kc] = et
                if kc > 0:
                    attn_v(kc - 1)
                if filler is not None:
                    next(filler, None)
            attn_v(SC - 1)
            if filler is not None:
                for _ in filler:
                    pass
            stage = stp.tile([HD + 1, S], f32r, tag="st", name=f"stage{h}")
            nc.vector.tensor_copy(stage[:], psO[0:HD + 1, :])
            nc.sync.dma_start(sxpair[ds(hh, 1), :], stage[ds(HD, 1), :])
            nc.sync.dma_start(rawt[ds(hh * HD, HD), :], stage[0:HD, :])

        sxpairs = {}

        def emit_norm(oc):
            sxpair = sxpairs[oc]
            # reciprocal_approx_fast (~2e-6 rel err) instead of the iterative
            # divide: HW runs DIVIDE at 8 cycles/elem, which the cost model
            # undercounts; the approx op runs at normal DVE rate. Sumexp is
            # in [1, ~4e3], far from the undefined edge cases. The f32
            # scratch hop gives the f32r operand a rounding producer.
            # borrow a long-dead QT slot: no dependency on current tiles
            scratch = qkp.tile([2 * H, S], f32,
                               tag=f"q{(oc + DC - 2) % DC}", name=f"rcs{oc}")
            nc.vector.reciprocal_approx_fast(
                out=scratch[:], in_=sxpair[:].bitcast(f32))
            nc.vector.tensor_copy(sxpair[:], scratch[:])
            psB = pp.tile([P, 2 * NH], f32, tag="ps", name=f"psB{oc}")
            for qh in range(2):
                nc.tensor.matmul(
                    psB[:, ds(qh * NH, NH)],
                    sel2[:], sxpair[:, ds(qh * NH, NH)],
                    start=True, stop=True)
            nc.vector.tensor_tensor(raws[oc][:], raws[oc][:], psB[:], ALU.mult)

        wo_order = [(oh, dc) for oh in range(2) for dc in range(DC)]
        wots = {}

        def load_wo(i):
            oh, dc = wo_order[i]
            t = wrp.tile([P, NH], f32r, tag="wr", name=f"wo{oh}_{dc}")
            nc.sync.dma_start(t[:], woT_d[ds(dc * P, P), ds(oh * NH, NH)])
            wots[(oh, dc)] = t

        emit_qk(0, "q", wqT_d, QT)
        emit_qk(0, "k", wkT_d, KT)
        for oc in range(DC):
            if oc == DC - 1:
                # prefetch the first output-projection weight tiles: their
                # DMAs land while the last heads run
                for i in range(3):
                    load_wo(i)
            rawt = rp.tile([P, S], f32r, tag=f"r{oc}")
            raws[oc] = rawt
            # per-pair sumexp tile: rows 0/1 receive the heads' denominator
            # rows; rows 2..31 stay at 1.0 (finite, zeroed by sel2)
            sxpair = sxq.tile([2 * H, S], f32r, tag="sx", name=f"sx{oc}")
            nc.vector.tensor_copy(
                sxpair[:], ones_t[0:2 * H, 0:1].to_broadcast((2 * H, S)))
            fq = qk_gen(oc + 1, "q", wqT_d, QT) if oc + 1 < DC else None
            emit_head(oc, 0, rawt, sxpair, filler=fq)
            fk = qk_gen(oc + 1, "k", wkT_d, KT) if oc + 1 < DC else None
            emit_head(oc, 1, rawt, sxpair, filler=fk)
            sxpairs[oc] = sxpair
            # normalize the PREVIOUS pair here: its recip/DMA chain completed
            # during this pair's heads, so PE hits psB with no stall
            if oc >= 1:
                emit_norm(oc - 1)

        emit_norm(DC - 1)

        # ---- output projection Y[s, o] ----
        for oh in range(2):
            psYs = [pp.tile([P, 2 * NH], f32, tag="ps", name=f"psY{oh}_{j}") for j in range(4)]
            for dc in range(DC):
                i = oh * DC + dc
                if i + 3 < len(wo_order):
                    load_wo(i + 3)
                wot = wots.pop((oh, dc))
                for sc in range(SC):
                    nc.tensor.matmul(
                        psYs[sc // 2][:, ds((sc % 2) * NH, NH)],
                        raws[dc][:, ds(sc * P, P)], wot[:],
                        start=(dc == 0), stop=(dc == DC - 1))
            for sc in range(SC):
                # reuse the (long dead) xT slots as 8-wide output staging
                yt = xp.tile([P, NH], f32, tag=f"x{sc}", name=f"yt{oh}_{sc}")
                src_ap = psYs[sc // 2][:, ds((sc % 2) * NH, NH)]
                if sc % 2 == 0:
                    nc.vector.tensor_copy(yt[:], src_ap)
                else:
                    nc.scalar.copy(yt[:], src_ap)
                nc.sync.dma_start(y_d[ds(sc * P, P), ds(oh * NH, NH)], yt[:])


def build_nc():
    nc = bacc.Bacc("TRN2", target_bir_lowering=False, debug=False,
                   enable_asserts=False, num_devices=NCORES)
    xT_d = nc.dram_tensor("xT", (D, S), f32r, kind="ExternalInput").ap()
    wqT_d = nc.dram_tensor("wqT", (DC, P, DC, P), f32r, kind="ExternalInput").ap()
    wkT_d = nc.dram_tensor("wkT", (DC, P, DC, P), f32r, kind="ExternalInput").ap()
    wvT_d = nc.dram_tensor("wvT", (D, D), f32r, kind="ExternalInput").ap()
    woT_d = nc.dram_tensor("woT", (D, D), f32r, kind="ExternalInput").ap()
    y_d = nc.dram_tensor("y", (S, D), f32, kind="ExternalOutput").ap()
    with tile.TileContext(nc) as tc:
        emit(tc, xT_d, wqT_d, wkT_d, wvT_d, woT_d, y_d)
    nc.compile()
    return nc


_NC_CACHE = None


def _get_nc():
    global _NC_CACHE
    if _NC_CACHE is None:
        _NC_CACHE = build_nc()
    return _NC_CACHE


def _block_qk(w):
    # wT[dc*P+p, oc*P+o] -> [oc, p, dc, o] so each per-oc stationary load is
    # a single DMA of contiguous 4KB-per-partition descriptors
    wT = np.asarray(w, np.float32).T
    return np.ascontiguousarray(
        wT.reshape(DC, P, DC, P).transpose(2, 1, 0, 3))


def make_in_maps(x, wq, wk, wv, wo):
    x = np.asarray(x, dtype=np.float32)
    wqT = _block_qk(wq)
    wkT = _block_qk(wk)
    wvT = np.ascontiguousarray(np.asarray(wv, np.float32).T)
    woT = np.ascontiguousarray(np.asarray(wo, np.float32).T)
    in_maps = []
    for b in range(B):
        in_maps.append({
            "xT": np.ascontiguousarray(x[b].T),
            "wqT": wqT, "wkT": wkT, "wvT": wvT, "woT": woT,
        })
    return in_maps


def kernel(x, wq, wk, wv, wo):
    nc = _get_nc()
    in_maps = make_in_maps(x, wq, wk, wv, wo)
    res = bass_utils.run_bass_kernel_spmd(nc, in_maps, core_ids=list(range(NCORES)))
    return np.stack([res.results[b]["y"] for b in range(B)], axis=0)
